# revision 83
# baseline (speedup 1.0000x reference)
"""Trainium2 Bass kernel for nn_CFHoTWrapper (sparse attention with adapter gate).

Sharding: tensor-parallel over attention heads across 8 NeuronCores.
Each core computes 4 query heads + its 1 KV head end-to-end (QKV proj,
RoPE, scores, softmax, AV, partial O-projection); the tiny adapter gate
is replicated on every core. Per-core partial outputs (bf16) are summed
on the host.

Softmax is computed without max-subtraction (scores are O(5) for these
shapes so exp() is safe in fp32), and the per-key gate bias is folded in
multiplicatively: exp(s + m + g[k]) = exp(s) * exp(m) * w[k] with
w = exp(gate_scale * gate).  w scales the V rows, and an extra all-w
column appended to V yields the softmax denominator from the same
matmul that computes the numerator.

RoPE's rotate-half is a fixed row permutation, done as a PE matmul with
a permutation matrix (sign folded into the sin table) instead of
SBUF->SBUF DMA, so the whole QKV+RoPE phase stays on PE/Vector.
Emission order keeps the PE continuously busy (TRN2 DVFS only reaches
full clock after ~3us of uninterrupted PE activity): adapter and KV
projections stream k-outer behind the hT DMA, score blocks for the
first q-head pair are emitted between the two Q projection passes so
the Scalar engine's exp() — the co-bottleneck — starts as early as
possible, and the block schedule ends on the cheapest chunk.
"""

import math
import os
from contextlib import ExitStack

import numpy as np
import ml_dtypes

import concourse.bass as bass
import concourse.tile as tile
from concourse import mybir
from concourse.masks import make_identity
from concourse.bass_utils import run_bass_kernel_spmd

BF16 = ml_dtypes.bfloat16
F32 = np.float32

S = 2048
D = 2048
HD = 64
NH = 32
NKV = 8
NCORES = 8
HLOC = NH // NCORES          # 4 query heads per core
P = 128
NT = S // P                  # 16 sequence tiles of 128
NCH = 4                      # 4 sequence chunks of 512
CH = 512
ALPHA = 0.995
MASK_NEG_THRESH = -80.0      # exp() underflows to 0 below this

LAST_RESULT = None           # BassKernelResults of the last run (for test.py)


def _analyze_mask(maskT):
    """Classify [keys=128 x q=128] blocks of maskT and dedup non-trivial
    multiplicative (exp) mask patterns. maskT is [S, S] (keys, q)."""
    mb = [[None] * NT for _ in range(NT)]
    patterns = []
    pat_index = {}
    for j in range(NT):
        for i in range(NT):
            blk = maskT[j * P:(j + 1) * P, i * P:(i + 1) * P]
            if (blk < MASK_NEG_THRESH).all():
                mb[j][i] = 'skip'
            elif (blk == 0.0).all():
                mb[j][i] = 'plain'
            else:
                pat = np.exp(np.minimum(blk, 80.0)).astype(BF16)
                key = pat.tobytes()
                if key not in pat_index:
                    pat_index[key] = len(patterns)
                    patterns.append(pat)
                mb[j][i] = pat_index[key]
    av_incl = [[j for j in range(NT) if mb[j][i] != 'skip'] for i in range(NT)]
    return mb, patterns, av_incl


def _split_sync_waits(nc):
    """This walrus build supports only ONE embedded sync wait per
    instruction; hoist extra waits onto preceding sequencer NoOps."""
    for f in nc.m.functions:
        for bb in f.blocks:
            insts = bb.instructions
            idx = 0
            while idx < len(insts):
                inst = insts[idx]
                si = inst.sync_info
                if si is not None and si.on_wait and len(si.on_wait) > 1:
                    waits = list(si.on_wait)
                    for w in waits[:-1]:
                        nop = mybir.InstNoOp(
                            name=nc.get_next_instruction_name(),
                            engine=inst.engine,
                            sync_info=mybir.SyncInfo(on_wait=[w], on_update=[]),
                            bass_nofuse=True,
                        )
                        nc.register_instruction(nop)
                        insts.insert(idx, nop)
                        idx += 1
                    inst.sync_info = mybir.SyncInfo(
                        on_wait=[waits[-1]], on_update=list(si.on_update))
                idx += 1


def _build_program(mb, n_pat, av_incl, field_scale, b2_scaled, gate_scale):
    nc = bass.Bass()
    dt = mybir.dt

    hT_d = nc.declare_dram_parameter("hT", [D, S], dt.bfloat16, isOutput=False)
    wq_d = nc.declare_dram_parameter("wq", [P, NT * HLOC * HD], dt.bfloat16, isOutput=False)
    wkv_d = nc.declare_dram_parameter("wkv", [P, NT * 2 * HD], dt.bfloat16, isOutput=False)
    wo_d = nc.declare_dram_parameter("wo", [P, 2 * D], dt.bfloat16, isOutput=False)
    w1a_d = nc.declare_dram_parameter("w1a", [P, NT * 64], dt.bfloat16, isOutput=False)
    w2_d = nc.declare_dram_parameter("w2", [64, 1], dt.bfloat16, isOutput=False)
    b1_d = nc.declare_dram_parameter("b1", [64, 1], dt.float32, isOutput=False)
    cos2q_d = nc.declare_dram_parameter("cos2q", [P, S], dt.bfloat16, isOutput=False)
    sin2q_d = nc.declare_dram_parameter("sin2q", [P, S], dt.bfloat16, isOutput=False)
    cosk_d = nc.declare_dram_parameter("cosk", [HD, S], dt.bfloat16, isOutput=False)
    sink_d = nc.declare_dram_parameter("sink", [HD, S], dt.bfloat16, isOutput=False)
    pq_d = nc.declare_dram_parameter("pq", [P, P], dt.bfloat16, isOutput=False)
    if n_pat:
        pm_d = nc.declare_dram_parameter("pmask", [n_pat, P, P], dt.bfloat16, isOutput=False)
    out_d = nc.declare_dram_parameter("out", [S, D], dt.bfloat16, isOutput=True)

    with tile.TileContext(nc) as tc, ExitStack() as ctx:
        pers = ctx.enter_context(tc.tile_pool(name="pers", bufs=1))
        psp = ctx.enter_context(tc.tile_pool(name="psum", bufs=4, space="PSUM"))

        # persistent (phase-C-lifetime) tiles; DMAs are emitted inside the
        # phb block below so small early-needed weights go first on the queue
        w2 = pers.tile([64, 1], dt.bfloat16)
        b1 = pers.tile([64, 1], dt.float32)
        pqm = pers.tile([P, P], dt.bfloat16)
        hT = pers.tile([P, NT, S], dt.bfloat16)
        wq = pers.tile([P, NT, HLOC * HD], dt.bfloat16)
        if n_pat:
            pmask = pers.tile([P, n_pat, P], dt.bfloat16)
        wo = pers.tile([P, 2, D], dt.bfloat16)
        ident = pers.tile([P, P], dt.bfloat16)
        make_identity(nc, ident)
        # HAM warm-up: tiny dependency-free matmuls (one reused psum tile,
        # no readers needed) fill the PE's DMA-wait gaps during the hT
        # stream so the clock gate stays at 8/8 - an idle window would
        # halve the PE clock for the next ~3.4us of real matmuls
        # (borrows one slot of the existing [1,CH] f32 "sc" psum class so
        # no extra PSUM bank is needed; the field pass rotates fine on 3)
        wps = psp.tile([1, CH], dt.float32, tag="sc", bufs=4, name="warm")

        def warm(n):
            for _ in range(n):
                nc.tensor.matmul(wps[0:1, 0:P], ident[0:1, 0:1],
                                 ident[0:1, :], start=True, stop=True)

        qt_pair = [pers.tile([P, S], dt.bfloat16, tag=f"qp{t}", name=f"qp{t}")
                   for t in range(HLOC // 2)]
        kt = pers.tile([HD, S], dt.bfloat16)
        ktp = pers.tile([P, S], dt.bfloat16)      # kt duplicated at base 64
        vaug = pers.tile([P, NT, HD + 1], dt.bfloat16)
        wcol = pers.tile([P, NT], dt.float32)
        cos2q = pers.tile([P, S], dt.bfloat16)
        sin2q = pers.tile([P, S], dt.bfloat16)

        phbw = ctx.enter_context(tc.tile_pool(name="phbw", bufs=2))
        with tc.tile_pool(name="phb", bufs=1) as phb:
            # ---- load order: small early-phase weights, then the hT
            # stream the k-outer passes trail, then later-phase tensors ----
            w1a = phb.tile([P, NT, 64], dt.bfloat16)
            nc.scalar.dma_start(out=w1a, in_=w1a_d[:, :])
            wkv = phb.tile([P, NT, 2 * HD], dt.bfloat16)
            nc.scalar.dma_start(out=wkv, in_=wkv_d[:, :])
            # hT split across both HWDGE queues (SP even k, ACT odd k) to
            # double the streaming bandwidth the k-outer passes trail
            nc.sync.dma_start(out=hT[:, 0, :], in_=hT_d[0:P, :])
            nc.sync.dma_start(out=hT[:, 2, :], in_=hT_d[2 * P:3 * P, :])
            nc.sync.dma_start(out=w2, in_=w2_d[:, :])
            nc.sync.dma_start(out=b1, in_=b1_d[:, :])
            nc.sync.dma_start(out=pqm, in_=pq_d[:, :])
            for k in range(NT):
                if k in (0, 2):
                    continue
                eng = nc.sync if k % 2 == 0 else nc.scalar
                eng.dma_start(out=hT[:, k, :], in_=hT_d[k * P:(k + 1) * P, :])
            cosk = phb.tile([HD, S], dt.bfloat16)
            nc.sync.dma_start(out=cosk, in_=cosk_d[:, :])
            sink = phb.tile([HD, S], dt.bfloat16)
            nc.sync.dma_start(out=sink, in_=sink_d[:, :])
            nc.scalar.dma_start(out=wq, in_=wq_d[:, :])
            nc.scalar.dma_start(out=cos2q, in_=cos2q_d[:, :])
            nc.scalar.dma_start(out=sin2q, in_=sin2q_d[:, :])
            if n_pat:
                for m in range(n_pat):
                    nc.sync.dma_start(out=pmask[:, m, :], in_=pm_d[m, :, :])
            nc.sync.dma_start(out=wo, in_=wo_d[:, :])

            # --- pass 1a: adapter hmidT = gelu(W1eff^T @ hT + b1), k-outer
            # so the 4 chunk accumulators trail the hT DMA tile stream ---
            accA = [psp.tile([64, CH], dt.float32, tag="sc", bufs=4,
                             name=f"accA{c}") for c in range(NCH)]
            warm(16)
            for k in range(NT):
                for c in range(NCH):
                    nc.tensor.matmul(accA[c], w1a[:, k, :],
                                     hT[:, k, c * CH:(c + 1) * CH],
                                     start=(k == 0), stop=(k == NT - 1))
                warm(3)
            hmT = phb.tile([64, S], dt.bfloat16)
            for c in range(NCH):
                # exact gelu(x) = 0.5 * x * (1 + erf(x / sqrt(2))), x = ps + b1
                pre = phbw.tile([64, CH], dt.float32, tag="pre")
                nc.vector.tensor_scalar(pre, accA[c], b1, None, mybir.AluOpType.add)
                er = phbw.tile([64, CH], dt.float32, tag="er")
                nc.scalar.activation(er, pre, mybir.ActivationFunctionType.Erf,
                                     bias=0.0, scale=1.0 / math.sqrt(2.0))
                nc.vector.tensor_scalar(er, er, 0.5, 0.5,
                                        mybir.AluOpType.mult, mybir.AluOpType.add)
                nc.vector.tensor_mul(hmT[:, c * CH:(c + 1) * CH], pre, er)

            # --- pass 1b: fused K+V projection, k-outer ---
            kraw = phb.tile([HD, S], dt.bfloat16)
            vt = phb.tile([HD, S], dt.bfloat16)
            accKV = [psp.tile([P, CH], dt.float32, tag="sc", bufs=4,
                              name=f"accKV{c}") for c in range(NCH)]
            for k in range(NT):
                for c in range(NCH):
                    nc.tensor.matmul(accKV[c], wkv[:, k, :],
                                     hT[:, k, c * CH:(c + 1) * CH],
                                     start=(k == 0), stop=(k == NT - 1))
            for c in range(NCH):
                csl = slice(c * CH, (c + 1) * CH)
                nc.vector.tensor_copy(kraw[:, csl], accKV[c][0:HD, :])
                nc.vector.tensor_copy(vt[:, csl], accKV[c][HD:P, :])

            # --- K RoPE: rotate-half via PE permutation matmul ---
            for c in range(NCH):
                csl = slice(c * CH, (c + 1) * CH)
                pmk = psp.tile([HD, CH], dt.float32, tag="od", bufs=2)
                nc.tensor.matmul(pmk, pqm[0:HD, 0:HD], kraw[:, csl],
                                 start=True, stop=True)
                t1k = phbw.tile([HD, CH], dt.bfloat16, tag="t1")
                nc.vector.tensor_mul(t1k, kraw[:, csl], cosk[:, csl])
                t2k = phbw.tile([HD, CH], dt.bfloat16, tag="t2")
                nc.vector.tensor_mul(t2k, pmk, sink[:, csl])
                nc.vector.tensor_add(kt[:, csl], t1k, t2k)
            nc.sync.dma_start(out=ktp[HD:P, :], in_=kt[:, :])

            # --- V tiles: PE transpose into unscaled vraw now (dep-free);
            # the wcol gate scaling runs later on GpSimd so the slow field
            # chain never stalls the in-order PE queue ---
            vraw = phbw.tile([P, NT, HD], dt.bfloat16, tag="vraw", bufs=1)
            for st in range(NT):
                pv = psp.tile([P, HD], dt.bfloat16, tag="od", bufs=2)
                nc.tensor.transpose(pv, vt[:, st * P:(st + 1) * P],
                                    ident[0:HD, 0:HD])
                nc.vector.tensor_copy(vraw[:, st, :], pv)

            # --- field row = field_scale * (hmidT^T @ W2 + b2); gate ---
            field = phb.tile([1, S], dt.float32)
            scratch = phb.tile([1, S], dt.float32)
            for c in range(NCH):
                ps = psp.tile([1, CH], dt.float32, tag="sc", bufs=4)
                nc.tensor.matmul(ps, w2, hmT[:, c * CH:(c + 1) * CH],
                                 start=True, stop=True)
                nc.vector.tensor_scalar(field[:, c * CH:(c + 1) * CH], ps,
                                        field_scale, b2_scaled,
                                        mybir.AluOpType.mult, mybir.AluOpType.add)
            ssum = phb.tile([1, 1], dt.float32)
            nc.vector.reduce_sum(ssum, field, axis=mybir.AxisListType.X)
            mean = phb.tile([1, 1], dt.float32)
            nc.vector.tensor_scalar_mul(mean, ssum, 1.0 / S)
            nc.vector.tensor_scalar(field, field, mean, None, mybir.AluOpType.subtract)
            nc.scalar.square(scratch, field)
            ss2 = phb.tile([1, 1], dt.float32)
            nc.vector.reduce_sum(ss2, scratch, axis=mybir.AxisListType.X)
            std = phb.tile([1, 1], dt.float32)
            nc.scalar.activation(std, ss2, mybir.ActivationFunctionType.Sqrt,
                                 bias=0.0, scale=1.0 / (S - 1))
            nc.vector.tensor_scalar_add(std, std, 1e-6)
            rstd = phb.tile([1, 1], dt.float32)
            nc.vector.reciprocal(rstd, std)
            gsr = phb.tile([1, 1], dt.float32)
            nc.vector.tensor_scalar_mul(gsr, rstd, gate_scale)
            # w row = exp(gate_scale * gate), into scratch
            nc.scalar.activation(scratch, field, mybir.ActivationFunctionType.Exp,
                                 bias=0.0, scale=gsr)
            # transpose the w row into per-partition columns [128, 16] via a
            # DRAM bounce (SBUF partitions are not element-addressable across
            # the partition stride, so an in-SBUF gather is illegal on HW)
            wrow_dram = nc.dram_tensor("wrow_dram", [1, S], dt.float32)
            nc.sync.dma_start(out=wrow_dram[:, :], in_=scratch)
            nc.sync.dma_start(out=wcol,
                              in_=wrow_dram[0, :].rearrange("(j p) -> p j", p=P))


        # ------------- phase C setup: attention emit helpers -------------
        with tc.tile_pool(name="phc", bufs=2) as phc, \
             tc.tile_pool(name="phcs", bufs=4) as phcs, \
             tc.tile_pool(name="phd", bufs=2) as phd:

            def chunk_start(j, c):
                for ii in range(4 * c, 4 * c + 4):
                    if mb[j][ii] != 'skip':
                        return (ii % 4) * P
                return None

            pts_store = {}
            attn = phcs.tile([P, NT, HLOC * HD], dt.bfloat16, tag="attn", bufs=1)

            def emit_scores(c, t):
                """Scores for head pair (2t, 2t+1): K=64 matmuls in PE
                row-groups 0 and 64 (A at base 0, B at base 64)."""
                ptsA, ptsB = {}, {}
                for j in range(NT):
                    s0 = chunk_start(j, c)
                    if s0 is None:
                        continue
                    jsl = slice(j * P, (j + 1) * P)
                    csl = slice(c * CH + s0, (c + 1) * CH)
                    psA = psp.tile([P, CH], dt.float32, tag="sc", bufs=4,
                                   name="ps_scA")
                    nc.tensor.matmul(psA[:, s0:CH], kt[:, jsl],
                                     qt_pair[t][0:HD, csl],
                                     start=True, stop=True, tile_position=(0, 0))
                    psB = psp.tile([P, CH], dt.float32, tag="sc", bufs=4,
                                   name="ps_scB")
                    nc.tensor.matmul(psB[:, s0:CH], ktp[HD:P, jsl],
                                     qt_pair[t][HD:P, csl],
                                     start=True, stop=True, tile_position=(64, 0))
                    for pts, ps, tagc in ((ptsA, psA, "pt"), (ptsB, psB, "pu")):
                        pt = phc.tile([P, CH], dt.bfloat16, tag=f"{tagc}{j}",
                                      name=f"{tagc}{j}")
                        nc.scalar.activation(pt[:, s0:CH], ps[:, s0:CH],
                                             mybir.ActivationFunctionType.Exp)
                        for ii in range(4 * c, 4 * c + 4):
                            kind = mb[j][ii]
                            if kind in ('skip', 'plain'):
                                continue
                            qq = slice((ii % 4) * P, (ii % 4 + 1) * P)
                            nc.vector.tensor_mul(pt[:, qq], pt[:, qq],
                                                 pmask[:, kind, :])
                        pts[j] = pt
                pts_store[(c, 2 * t)] = ptsA
                pts_store[(c, 2 * t + 1)] = ptsB

            def emit_av(c, h):
                pts = pts_store.pop((c, h))
                for i in range(4 * c, 4 * c + 4):
                    js = av_incl[i]
                    hsl = slice(h * HD, (h + 1) * HD)
                    if not js:
                        nc.vector.memset(attn[:, i, hsl], 0.0)
                        continue
                    pa = psp.tile([P, HD + 1], dt.float32, tag="av", bufs=2,
                                  name="ps_av")
                    qq = slice((i % 4) * P, (i % 4 + 1) * P)
                    for idx, j in enumerate(js):
                        nc.tensor.matmul(pa, pts[j][:, qq], vaug[:, j, :],
                                         start=(idx == 0),
                                         stop=(idx == len(js) - 1))
                    rc = phcs.tile([P, 1], dt.float32, tag="rc", name="rc")
                    nc.vector.reciprocal(rc, pa[:, HD:HD + 1])
                    nc.vector.tensor_scalar(attn[:, i, hsl], pa[:, 0:HD], rc,
                                            None, mybir.AluOpType.mult)

            def emit_oproj(c, tail=False, i_list=None):
                if i_list is None:
                    i_list = range(4 * c, 4 * c + 4)
                for i in i_list:
                    aTs = []
                    for g in range(2):
                        ptr = psp.tile([P, P], dt.bfloat16, tag="od", bufs=2,
                                       name="ptr")
                        nc.tensor.transpose(ptr, attn[:, i, g * P:(g + 1) * P],
                                            ident)
                        aT = phcs.tile([P, P], dt.bfloat16, tag="aT", name="aT")
                        if tail:
                            nc.scalar.copy(aT, ptr)
                        else:
                            nc.vector.tensor_copy(aT, ptr)
                        aTs.append(aT)
                    ot = phd.tile([P, D], dt.bfloat16, tag="outsb", name="ot")
                    for dc in range(NCH):
                        po = psp.tile([P, CH], dt.float32, tag="od", bufs=2,
                                      name="ps_o")
                        dsl = slice(dc * CH, (dc + 1) * CH)
                        nc.tensor.matmul(po, aTs[0], wo[:, 0, dsl],
                                         start=True, stop=False)
                        nc.tensor.matmul(po, aTs[1], wo[:, 1, dsl],
                                         start=False, stop=True)
                        if tail and dc % 2 == 1:
                            nc.scalar.copy(ot[:, dsl], po)
                        else:
                            nc.vector.tensor_copy(ot[:, dsl], po)
                    if i == 3:
                        # final block: halve the tail DMA across both queues
                        nc.sync.dma_start(out=out_d[i * P:(i + 1) * P, 0:D // 2],
                                          in_=ot[:, 0:D // 2])
                        nc.scalar.dma_start(out=out_d[i * P:(i + 1) * P, D // 2:D],
                                            in_=ot[:, D // 2:D])
                    else:
                        eng = nc.sync if i % 2 == 0 else nc.scalar
                        eng.dma_start(out=out_d[i * P:(i + 1) * P, :], in_=ot)

            # --- pass 2: Q projections (chunk-major, hT now resident),
            # RoPE inline; first-pair score blocks emitted between the
            # two passes so exp() starts as early as possible ---
            for t in range(HLOC // 2):
                # chunks ordered so the first score blocks' q-chunks rope first
                for c in ([1, 2, 3, 0] if t == 0 else [3, 2, 1, 0]):
                    csl = slice(c * CH, (c + 1) * CH)
                    psq = psp.tile([P, CH], dt.float32, tag="sc", bufs=4,
                                   name="ps_q")
                    for k in range(NT):
                        nc.tensor.matmul(psq, wq[:, k, t * P:(t + 1) * P],
                                         hT[:, k, csl],
                                         start=(k == 0), stop=(k == NT - 1))
                    pair_c = phbw.tile([P, CH], dt.bfloat16, tag="pair")
                    nc.vector.tensor_copy(pair_c, psq)
                    pmq = psp.tile([P, CH], dt.float32, tag="od", bufs=2)
                    nc.tensor.matmul(pmq, pqm, pair_c, start=True, stop=True)
                    t1 = phbw.tile([P, CH], dt.bfloat16, tag="t1")
                    nc.vector.tensor_mul(t1, pair_c, cos2q[:, csl])
                    t2 = phbw.tile([P, CH], dt.bfloat16, tag="t2")
                    nc.vector.tensor_mul(t2, pmq, sin2q[:, csl])
                    nc.vector.tensor_add(qt_pair[t][:, csl], t1, t2)
                if t == 0:
                    emit_scores(1, 0)
                    emit_scores(2, 0)
                    # gate scaling + denominator column; emitted after the
                    # t=0 ropes so the wcol DRAM bounce never stalls them
                    for st in range(NT):
                        nc.vector.tensor_scalar(vaug[:, st, 0:HD],
                                                vraw[:, st, :],
                                                wcol[:, st:st + 1], None,
                                                mybir.AluOpType.mult)
                        nc.vector.tensor_copy(vaug[:, st, HD:HD + 1],
                                              wcol[:, st:st + 1])

            # interleaved block schedule: exp-heavy blocks mid-stream,
            # cheapest chunk (c=0) last to minimize the drain tail.
            seq = [("av", 1, 0), ("sc", 3, 0), ("av", 2, 0), ("sc", 3, 1),
                   ("av", 3, 0), ("sc", 1, 1), ("av", 1, 1), ("sc", 2, 1),
                   ("av", 2, 1), ("sc", 0, 0), ("av", 3, 1), ("sc", 0, 1),
                   ("av", 0, 0), ("av", 0, 1)]
            # each chunk's oproj is split in two: the second half is
            # deferred to the start of the NEXT schedule unit, threading
            # dense real PE work into the exp-paced (and otherwise
            # gap-prone) head of that unit instead of one 8us burst
            pair_done = {c: 0 for c in range(NCH)}
            pending = []
            for kind, c, t in seq:
                if pending:
                    pending.pop(0)()
                if kind == "sc":
                    emit_scores(c, t)
                else:
                    emit_av(c, 2 * t)
                    emit_av(c, 2 * t + 1)
                    pair_done[c] += 1
                    if pair_done[c] == HLOC // 2:
                        emit_oproj(c, tail=(c != 1),
                                   i_list=range(4 * c, 4 * c + 2))
                        pending.append(
                            lambda cc=c: emit_oproj(
                                cc, tail=(cc != 1),
                                i_list=range(4 * cc + 2, 4 * cc + 4)))
            for f in pending:
                f()

    _split_sync_waits(nc)
    return nc


def kernel(**inputs):
    global LAST_RESULT
    inp = {k: np.asarray(v) for k, v in inputs.items()}
    h = inp["hidden_states"].astype(F32).reshape(S, D)
    mask = inp["attention_mask"].astype(F32).reshape(S, S)
    cos = inp["cos"].astype(F32)
    sin = inp["sin"].astype(F32)
    Wf = inp["Wf"].astype(F32)
    W1 = inp["W1"].astype(F32)
    b1 = inp["b1"].astype(F32)
    W2 = inp["W2"].astype(F32)
    b2 = float(inp["b2"].reshape(-1)[0])
    gate_scale = float(inp["gate_scale"])
    Wq = inp["Wq"].astype(F32)
    Wk = inp["Wk"].astype(F32)
    Wv = inp["Wv"].astype(F32)
    Wo = inp["Wo"].astype(F32)

    maskT = np.ascontiguousarray(mask.T)
    mb, patterns, av_incl = _analyze_mask(maskT)
    n_pat = len(patterns)
    assert n_pat <= 64, f"too many unique mask patterns ({n_pat})"

    field_scale = float(F32(1.0 - ALPHA))
    b2_scaled = float(F32(b2) * F32(field_scale))

    nc = _build_program(mb, n_pat, av_incl, field_scale, b2_scaled, gate_scale)

    # host-side shared tensors
    hT = np.ascontiguousarray(h.T).astype(BF16)
    cosT = np.ascontiguousarray(cos.T)                       # [64, S]
    sinT = np.ascontiguousarray(sin.T)
    sin_signed = sinT.copy()
    sin_signed[0:32] = -sin_signed[0:32]
    inv_sqrt_hd = 1.0 / math.sqrt(HD)
    cos2q = np.vstack([cosT, cosT]) * inv_sqrt_hd            # [128, S]
    sin2q = np.vstack([sin_signed, sin_signed]) * inv_sqrt_hd
    cosk = cosT.astype(BF16)
    sink = sin_signed.astype(BF16)
    # rotate-half permutation (swap 32-row halves within each 64-row head)
    pq = np.zeros((P, P), dtype=BF16)
    for m in range(P):
        base = (m // HD) * HD
        r = m - base
        src = base + (r + 32) % HD
        pq[src, m] = 1.0
    w1a = (W1[:D].astype(np.float64)
           + Wf.astype(np.float64) @ W1[D:].astype(np.float64)).astype(F32).astype(BF16)

    def kmajor(w):
        # [D, F] -> [128, NT*F]: partition-major so the SBUF load is contiguous
        f = w.shape[1]
        return np.ascontiguousarray(
            w.reshape(NT, P, f).transpose(1, 0, 2).reshape(P, NT * f))

    w1a = kmajor(w1a)
    w2 = W2.reshape(64, 1).astype(BF16)
    b1c = b1.reshape(64, 1).astype(F32)
    pm = np.stack(patterns) if n_pat else None

    in_maps = []
    for c in range(NCORES):
        m = {
            "hT": hT,
            "wq": kmajor(Wq[:, c * HLOC * HD:(c + 1) * HLOC * HD].astype(BF16)),
            "wkv": kmajor(np.concatenate(
                [Wk[:, c * HD:(c + 1) * HD], Wv[:, c * HD:(c + 1) * HD]],
                axis=1).astype(BF16)),
            "wo": np.ascontiguousarray(
                Wo[c * HLOC * HD:(c + 1) * HLOC * HD, :].astype(BF16)
                .reshape(2, P, D).transpose(1, 0, 2).reshape(P, 2 * D)),
            "w1a": w1a, "w2": w2, "b1": b1c,
            "cos2q": cos2q.astype(BF16), "sin2q": sin2q.astype(BF16),
            "cosk": cosk, "sink": sink, "pq": pq,
        }
        if n_pat:
            m["pmask"] = pm
        in_maps.append(m)

    trace = False
    if os.environ.get("KERNEL_TRACE"):
        try:
            import antenv.axon_hooks  # noqa: F401  (profiling shim, dev only)
            trace = True
        except ImportError:
            pass

    res = run_bass_kernel_spmd(nc, in_maps, list(range(NCORES)), trace=trace)
    LAST_RESULT = res

    out = np.zeros((S, D), dtype=F32)
    for c in range(NCORES):
        out += res.results[c]["out"].astype(F32)
    return out.reshape(1, S, D)



# revision 84
# speedup vs baseline: 1.0080x; 1.0080x over previous
"""Trainium2 Bass kernel for nn_CFHoTWrapper (sparse attention with adapter gate).

Sharding: tensor-parallel over attention heads across 8 NeuronCores.
Each core computes 4 query heads + its 1 KV head end-to-end (QKV proj,
RoPE, scores, softmax, AV, partial O-projection); the tiny adapter gate
is replicated on every core. Per-core partial outputs (bf16) are summed
on the host.

Softmax is computed without max-subtraction (scores are O(5) for these
shapes so exp() is safe in fp32), and the per-key gate bias is folded in
multiplicatively: exp(s + m + g[k]) = exp(s) * exp(m) * w[k] with
w = exp(gate_scale * gate).  w scales the V rows, and an extra all-w
column appended to V yields the softmax denominator from the same
matmul that computes the numerator.

RoPE's rotate-half is a fixed row permutation, done as a PE matmul with
a permutation matrix (sign folded into the sin table) instead of
SBUF->SBUF DMA, so the whole QKV+RoPE phase stays on PE/Vector.
Emission order keeps the PE continuously busy (TRN2 DVFS only reaches
full clock after ~3us of uninterrupted PE activity): adapter and KV
projections stream k-outer behind the hT DMA, score blocks for the
first q-head pair are emitted between the two Q projection passes so
the Scalar engine's exp() — the co-bottleneck — starts as early as
possible, and the block schedule ends on the cheapest chunk.
"""

import math
import os
from contextlib import ExitStack

import numpy as np
import ml_dtypes

import concourse.bass as bass
import concourse.tile as tile
from concourse import mybir
from concourse.masks import make_identity
from concourse.bass_utils import run_bass_kernel_spmd

BF16 = ml_dtypes.bfloat16
F32 = np.float32

S = 2048
D = 2048
HD = 64
NH = 32
NKV = 8
NCORES = 8
HLOC = NH // NCORES          # 4 query heads per core
P = 128
NT = S // P                  # 16 sequence tiles of 128
NCH = 4                      # 4 sequence chunks of 512
CH = 512
ALPHA = 0.995
MASK_NEG_THRESH = -80.0      # exp() underflows to 0 below this

LAST_RESULT = None           # BassKernelResults of the last run (for test.py)


def _analyze_mask(maskT):
    """Classify [keys=128 x q=128] blocks of maskT and dedup non-trivial
    multiplicative (exp) mask patterns. maskT is [S, S] (keys, q)."""
    mb = [[None] * NT for _ in range(NT)]
    patterns = []
    pat_index = {}
    for j in range(NT):
        for i in range(NT):
            blk = maskT[j * P:(j + 1) * P, i * P:(i + 1) * P]
            if (blk < MASK_NEG_THRESH).all():
                mb[j][i] = 'skip'
            elif (blk == 0.0).all():
                mb[j][i] = 'plain'
            else:
                pat = np.exp(np.minimum(blk, 80.0)).astype(BF16)
                key = pat.tobytes()
                if key not in pat_index:
                    pat_index[key] = len(patterns)
                    patterns.append(pat)
                mb[j][i] = pat_index[key]
    av_incl = [[j for j in range(NT) if mb[j][i] != 'skip'] for i in range(NT)]
    return mb, patterns, av_incl


def _split_sync_waits(nc):
    """This walrus build supports only ONE embedded sync wait per
    instruction; hoist extra waits onto preceding sequencer NoOps."""
    for f in nc.m.functions:
        for bb in f.blocks:
            insts = bb.instructions
            idx = 0
            while idx < len(insts):
                inst = insts[idx]
                si = inst.sync_info
                if si is not None and si.on_wait and len(si.on_wait) > 1:
                    waits = list(si.on_wait)
                    for w in waits[:-1]:
                        nop = mybir.InstNoOp(
                            name=nc.get_next_instruction_name(),
                            engine=inst.engine,
                            sync_info=mybir.SyncInfo(on_wait=[w], on_update=[]),
                            bass_nofuse=True,
                        )
                        nc.register_instruction(nop)
                        insts.insert(idx, nop)
                        idx += 1
                    inst.sync_info = mybir.SyncInfo(
                        on_wait=[waits[-1]], on_update=list(si.on_update))
                idx += 1


def _build_program(mb, n_pat, av_incl, field_scale, b2_scaled, gate_scale):
    nc = bass.Bass()
    dt = mybir.dt

    hT_d = nc.declare_dram_parameter("hT", [D, S], dt.bfloat16, isOutput=False)
    wq_d = nc.declare_dram_parameter("wq", [P, NT * HLOC * HD], dt.bfloat16, isOutput=False)
    wkv_d = nc.declare_dram_parameter("wkv", [P, NT * 2 * HD], dt.bfloat16, isOutput=False)
    wo_d = nc.declare_dram_parameter("wo", [P, 2 * D], dt.bfloat16, isOutput=False)
    w1a_d = nc.declare_dram_parameter("w1a", [P, NT * 64], dt.bfloat16, isOutput=False)
    w2_d = nc.declare_dram_parameter("w2", [64, 1], dt.bfloat16, isOutput=False)
    b1_d = nc.declare_dram_parameter("b1", [64, 1], dt.float32, isOutput=False)
    cos2q_d = nc.declare_dram_parameter("cos2q", [P, S], dt.bfloat16, isOutput=False)
    sin2q_d = nc.declare_dram_parameter("sin2q", [P, S], dt.bfloat16, isOutput=False)
    cosk_d = nc.declare_dram_parameter("cosk", [HD, S], dt.bfloat16, isOutput=False)
    sink_d = nc.declare_dram_parameter("sink", [HD, S], dt.bfloat16, isOutput=False)
    pq_d = nc.declare_dram_parameter("pq", [P, P], dt.bfloat16, isOutput=False)
    if n_pat:
        pm_d = nc.declare_dram_parameter("pmask", [n_pat, P, P], dt.bfloat16, isOutput=False)
    out_d = nc.declare_dram_parameter("out", [S, D], dt.bfloat16, isOutput=True)

    with tile.TileContext(nc) as tc, ExitStack() as ctx:
        pers = ctx.enter_context(tc.tile_pool(name="pers", bufs=1))
        psp = ctx.enter_context(tc.tile_pool(name="psum", bufs=4, space="PSUM"))

        # persistent (phase-C-lifetime) tiles; DMAs are emitted inside the
        # phb block below so small early-needed weights go first on the queue
        w2 = pers.tile([64, 1], dt.bfloat16)
        b1 = pers.tile([64, 1], dt.float32)
        pqm = pers.tile([P, P], dt.bfloat16)
        hT = pers.tile([P, NT, S], dt.bfloat16)
        wq = pers.tile([P, NT, HLOC * HD], dt.bfloat16)
        if n_pat:
            pmask = pers.tile([P, n_pat, P], dt.bfloat16)
        wo = pers.tile([P, 2, D], dt.bfloat16)
        ident = pers.tile([P, P], dt.bfloat16)
        make_identity(nc, ident)
        # HAM warm-up: tiny dependency-free matmuls (one reused psum tile,
        # no readers needed) fill the PE's DMA-wait gaps during the hT
        # stream so the clock gate stays at 8/8 - an idle window would
        # halve the PE clock for the next ~3.4us of real matmuls
        # (borrows one slot of the existing [1,CH] f32 "sc" psum class so
        # no extra PSUM bank is needed; the field pass rotates fine on 3)
        wps = psp.tile([1, CH], dt.float32, tag="sc", bufs=4, name="warm")

        def warm(n):
            for _ in range(n):
                nc.tensor.matmul(wps[0:1, 0:P], ident[0:1, 0:1],
                                 ident[0:1, :], start=True, stop=True)

        qt_pair = [pers.tile([P, S], dt.bfloat16, tag=f"qp{t}", name=f"qp{t}")
                   for t in range(HLOC // 2)]
        kt = pers.tile([HD, S], dt.bfloat16)
        ktp = pers.tile([P, S], dt.bfloat16)      # kt duplicated at base 64
        vaug = pers.tile([P, NT, HD + 1], dt.bfloat16)
        wcol = pers.tile([P, NT], dt.float32)
        cos2q = pers.tile([P, S], dt.bfloat16)
        sin2q = pers.tile([P, S], dt.bfloat16)

        phbw = ctx.enter_context(tc.tile_pool(name="phbw", bufs=2))
        with tc.tile_pool(name="phb", bufs=1) as phb:
            # ---- load order: small early-phase weights, then the hT
            # stream the k-outer passes trail, then later-phase tensors ----
            w1a = phb.tile([P, NT, 64], dt.bfloat16)
            nc.scalar.dma_start(out=w1a, in_=w1a_d[:, :])
            wkv = phb.tile([P, NT, 2 * HD], dt.bfloat16)
            nc.scalar.dma_start(out=wkv, in_=wkv_d[:, :])
            # hT split across both HWDGE queues (SP even k, ACT odd k) to
            # double the streaming bandwidth the k-outer passes trail
            nc.sync.dma_start(out=hT[:, 0, :], in_=hT_d[0:P, :])
            nc.sync.dma_start(out=hT[:, 2, :], in_=hT_d[2 * P:3 * P, :])
            nc.sync.dma_start(out=w2, in_=w2_d[:, :])
            nc.sync.dma_start(out=b1, in_=b1_d[:, :])
            nc.sync.dma_start(out=pqm, in_=pq_d[:, :])
            for k in range(NT):
                if k in (0, 2):
                    continue
                eng = nc.sync if k % 2 == 0 else nc.scalar
                eng.dma_start(out=hT[:, k, :], in_=hT_d[k * P:(k + 1) * P, :])
            cosk = phb.tile([HD, S], dt.bfloat16)
            nc.sync.dma_start(out=cosk, in_=cosk_d[:, :])
            sink = phb.tile([HD, S], dt.bfloat16)
            nc.sync.dma_start(out=sink, in_=sink_d[:, :])
            nc.scalar.dma_start(out=wq, in_=wq_d[:, :])
            nc.scalar.dma_start(out=cos2q, in_=cos2q_d[:, :])
            nc.scalar.dma_start(out=sin2q, in_=sin2q_d[:, :])
            if n_pat:
                for m in range(n_pat):
                    nc.sync.dma_start(out=pmask[:, m, :], in_=pm_d[m, :, :])
            nc.sync.dma_start(out=wo, in_=wo_d[:, :])

            # --- pass 1a: adapter hmidT = gelu(W1eff^T @ hT + b1), k-outer
            # so the 4 chunk accumulators trail the hT DMA tile stream ---
            accA = [psp.tile([64, CH], dt.float32, tag="sc", bufs=4,
                             name=f"accA{c}") for c in range(NCH)]
            warm(16)
            for k in range(NT):
                for c in range(NCH):
                    nc.tensor.matmul(accA[c], w1a[:, k, :],
                                     hT[:, k, c * CH:(c + 1) * CH],
                                     start=(k == 0), stop=(k == NT - 1))
                warm(3)
            hmT = phb.tile([64, S], dt.bfloat16)
            for c in range(NCH):
                # exact gelu(x) = 0.5 * x * (1 + erf(x / sqrt(2))), x = ps + b1
                pre = phbw.tile([64, CH], dt.float32, tag="pre")
                nc.vector.tensor_scalar(pre, accA[c], b1, None, mybir.AluOpType.add)
                er = phbw.tile([64, CH], dt.float32, tag="er")
                nc.scalar.activation(er, pre, mybir.ActivationFunctionType.Erf,
                                     bias=0.0, scale=1.0 / math.sqrt(2.0))
                nc.vector.tensor_scalar(er, er, 0.5, 0.5,
                                        mybir.AluOpType.mult, mybir.AluOpType.add)
                nc.vector.tensor_mul(hmT[:, c * CH:(c + 1) * CH], pre, er)

            # --- pass 1b: fused K+V projection, k-outer ---
            kraw = phb.tile([HD, S], dt.bfloat16)
            vt = phb.tile([HD, S], dt.bfloat16)
            accKV = [psp.tile([P, CH], dt.float32, tag="sc", bufs=4,
                              name=f"accKV{c}") for c in range(NCH)]
            for k in range(NT):
                for c in range(NCH):
                    nc.tensor.matmul(accKV[c], wkv[:, k, :],
                                     hT[:, k, c * CH:(c + 1) * CH],
                                     start=(k == 0), stop=(k == NT - 1))
            for c in range(NCH):
                csl = slice(c * CH, (c + 1) * CH)
                nc.vector.tensor_copy(kraw[:, csl], accKV[c][0:HD, :])
                nc.vector.tensor_copy(vt[:, csl], accKV[c][HD:P, :])

            # --- K RoPE: rotate-half via PE permutation matmul ---
            for c in range(NCH):
                csl = slice(c * CH, (c + 1) * CH)
                pmk = psp.tile([HD, CH], dt.float32, tag="od", bufs=2)
                nc.tensor.matmul(pmk, pqm[0:HD, 0:HD], kraw[:, csl],
                                 start=True, stop=True)
                t1k = phbw.tile([HD, CH], dt.bfloat16, tag="t1")
                nc.vector.tensor_mul(t1k, kraw[:, csl], cosk[:, csl])
                t2k = phbw.tile([HD, CH], dt.bfloat16, tag="t2")
                nc.vector.tensor_mul(t2k, pmk, sink[:, csl])
                nc.vector.tensor_add(kt[:, csl], t1k, t2k)
            nc.sync.dma_start(out=ktp[HD:P, :], in_=kt[:, :])

            # --- V tiles: PE transpose into unscaled vraw now (dep-free);
            # the wcol gate scaling runs later on GpSimd so the slow field
            # chain never stalls the in-order PE queue ---
            vraw = phbw.tile([P, NT, HD], dt.bfloat16, tag="vraw", bufs=1)
            for st in range(NT):
                pv = psp.tile([P, HD], dt.bfloat16, tag="od", bufs=2)
                nc.tensor.transpose(pv, vt[:, st * P:(st + 1) * P],
                                    ident[0:HD, 0:HD])
                nc.vector.tensor_copy(vraw[:, st, :], pv)

            # --- field row = field_scale * (hmidT^T @ W2 + b2); gate ---
            field = phb.tile([1, S], dt.float32)
            scratch = phb.tile([1, S], dt.float32)
            for c in range(NCH):
                ps = psp.tile([1, CH], dt.float32, tag="sc", bufs=4)
                nc.tensor.matmul(ps, w2, hmT[:, c * CH:(c + 1) * CH],
                                 start=True, stop=True)
                nc.vector.tensor_scalar(field[:, c * CH:(c + 1) * CH], ps,
                                        field_scale, b2_scaled,
                                        mybir.AluOpType.mult, mybir.AluOpType.add)
            ssum = phb.tile([1, 1], dt.float32)
            nc.vector.reduce_sum(ssum, field, axis=mybir.AxisListType.X)
            mean = phb.tile([1, 1], dt.float32)
            nc.vector.tensor_scalar_mul(mean, ssum, 1.0 / S)
            nc.vector.tensor_scalar(field, field, mean, None, mybir.AluOpType.subtract)
            nc.scalar.square(scratch, field)
            ss2 = phb.tile([1, 1], dt.float32)
            nc.vector.reduce_sum(ss2, scratch, axis=mybir.AxisListType.X)
            std = phb.tile([1, 1], dt.float32)
            nc.scalar.activation(std, ss2, mybir.ActivationFunctionType.Sqrt,
                                 bias=0.0, scale=1.0 / (S - 1))
            nc.vector.tensor_scalar_add(std, std, 1e-6)
            rstd = phb.tile([1, 1], dt.float32)
            nc.vector.reciprocal(rstd, std)
            gsr = phb.tile([1, 1], dt.float32)
            nc.vector.tensor_scalar_mul(gsr, rstd, gate_scale)
            # w row = exp(gate_scale * gate), into scratch
            nc.scalar.activation(scratch, field, mybir.ActivationFunctionType.Exp,
                                 bias=0.0, scale=gsr)
            # transpose the w row into per-partition columns [128, 16] via a
            # DRAM bounce (SBUF partitions are not element-addressable across
            # the partition stride, so an in-SBUF gather is illegal on HW)
            wrow_dram = nc.dram_tensor("wrow_dram", [1, S], dt.float32)
            nc.sync.dma_start(out=wrow_dram[:, :], in_=scratch)
            nc.sync.dma_start(out=wcol,
                              in_=wrow_dram[0, :].rearrange("(j p) -> p j", p=P))


        # ------------- phase C setup: attention emit helpers -------------
        with tc.tile_pool(name="phc", bufs=2) as phc, \
             tc.tile_pool(name="phcs", bufs=4) as phcs, \
             tc.tile_pool(name="phd", bufs=2) as phd:

            def chunk_start(j, c):
                for ii in range(4 * c, 4 * c + 4):
                    if mb[j][ii] != 'skip':
                        return (ii % 4) * P
                return None

            pts_store = {}
            attn = phcs.tile([P, NT, HLOC * HD], dt.bfloat16, tag="attn", bufs=1)

            def emit_scores(c, t):
                """Scores for head pair (2t, 2t+1): K=64 matmuls in PE
                row-groups 0 and 64 (A at base 0, B at base 64)."""
                ptsA, ptsB = {}, {}
                for j in range(NT):
                    s0 = chunk_start(j, c)
                    if s0 is None:
                        continue
                    jsl = slice(j * P, (j + 1) * P)
                    csl = slice(c * CH + s0, (c + 1) * CH)
                    psA = psp.tile([P, CH], dt.float32, tag="sc", bufs=4,
                                   name="ps_scA")
                    nc.tensor.matmul(psA[:, s0:CH], kt[:, jsl],
                                     qt_pair[t][0:HD, csl],
                                     start=True, stop=True, tile_position=(0, 0))
                    psB = psp.tile([P, CH], dt.float32, tag="sc", bufs=4,
                                   name="ps_scB")
                    nc.tensor.matmul(psB[:, s0:CH], ktp[HD:P, jsl],
                                     qt_pair[t][HD:P, csl],
                                     start=True, stop=True, tile_position=(64, 0))
                    for pts, ps, tagc in ((ptsA, psA, "pt"), (ptsB, psB, "pu")):
                        pt = phc.tile([P, CH], dt.bfloat16, tag=f"{tagc}{j}",
                                      name=f"{tagc}{j}")
                        nc.scalar.activation(pt[:, s0:CH], ps[:, s0:CH],
                                             mybir.ActivationFunctionType.Exp)
                        for ii in range(4 * c, 4 * c + 4):
                            kind = mb[j][ii]
                            if kind in ('skip', 'plain'):
                                continue
                            qq = slice((ii % 4) * P, (ii % 4 + 1) * P)
                            nc.vector.tensor_mul(pt[:, qq], pt[:, qq],
                                                 pmask[:, kind, :])
                        pts[j] = pt
                pts_store[(c, 2 * t)] = ptsA
                pts_store[(c, 2 * t + 1)] = ptsB

            def emit_av(c, h):
                pts = pts_store.pop((c, h))
                for i in range(4 * c, 4 * c + 4):
                    js = av_incl[i]
                    hsl = slice(h * HD, (h + 1) * HD)
                    if not js:
                        nc.vector.memset(attn[:, i, hsl], 0.0)
                        continue
                    pa = psp.tile([P, HD + 1], dt.float32, tag="av", bufs=2,
                                  name="ps_av")
                    qq = slice((i % 4) * P, (i % 4 + 1) * P)
                    for idx, j in enumerate(js):
                        nc.tensor.matmul(pa, pts[j][:, qq], vaug[:, j, :],
                                         start=(idx == 0),
                                         stop=(idx == len(js) - 1))
                    rc = phcs.tile([P, 1], dt.float32, tag="rc", name="rc")
                    nc.vector.reciprocal(rc, pa[:, HD:HD + 1])
                    nc.vector.tensor_scalar(attn[:, i, hsl], pa[:, 0:HD], rc,
                                            None, mybir.AluOpType.mult)

            def emit_oproj(c, tail=False, i_list=None):
                if i_list is None:
                    i_list = range(4 * c, 4 * c + 4)
                for i in i_list:
                    aTs = []
                    for g in range(2):
                        ptr = psp.tile([P, P], dt.bfloat16, tag="od", bufs=2,
                                       name="ptr")
                        nc.tensor.transpose(ptr, attn[:, i, g * P:(g + 1) * P],
                                            ident)
                        aT = phcs.tile([P, P], dt.bfloat16, tag="aT", name="aT")
                        if tail:
                            nc.scalar.copy(aT, ptr)
                        else:
                            nc.vector.tensor_copy(aT, ptr)
                        aTs.append(aT)
                    ot = phd.tile([P, D], dt.bfloat16, tag="outsb", name="ot")
                    for dc in range(NCH):
                        po = psp.tile([P, CH], dt.float32, tag="od", bufs=2,
                                      name="ps_o")
                        dsl = slice(dc * CH, (dc + 1) * CH)
                        nc.tensor.matmul(po, aTs[0], wo[:, 0, dsl],
                                         start=True, stop=False)
                        nc.tensor.matmul(po, aTs[1], wo[:, 1, dsl],
                                         start=False, stop=True)
                        if tail and dc % 2 == 1:
                            nc.scalar.copy(ot[:, dsl], po)
                        else:
                            nc.vector.tensor_copy(ot[:, dsl], po)
                    nc.sync.dma_start(out=out_d[i * P:(i + 1) * P, :], in_=ot)

            # --- pass 2: Q projections (chunk-major, hT now resident),
            # RoPE inline; first-pair score blocks emitted between the
            # two passes so exp() starts as early as possible ---
            for t in range(HLOC // 2):
                # chunks ordered so the first score blocks' q-chunks rope first
                for c in ([1, 2, 3, 0] if t == 0 else [3, 2, 1, 0]):
                    csl = slice(c * CH, (c + 1) * CH)
                    psq = psp.tile([P, CH], dt.float32, tag="sc", bufs=4,
                                   name="ps_q")
                    for k in range(NT):
                        nc.tensor.matmul(psq, wq[:, k, t * P:(t + 1) * P],
                                         hT[:, k, csl],
                                         start=(k == 0), stop=(k == NT - 1))
                    pair_c = phbw.tile([P, CH], dt.bfloat16, tag="pair")
                    nc.vector.tensor_copy(pair_c, psq)
                    pmq = psp.tile([P, CH], dt.float32, tag="od", bufs=2)
                    nc.tensor.matmul(pmq, pqm, pair_c, start=True, stop=True)
                    t1 = phbw.tile([P, CH], dt.bfloat16, tag="t1")
                    nc.vector.tensor_mul(t1, pair_c, cos2q[:, csl])
                    t2 = phbw.tile([P, CH], dt.bfloat16, tag="t2")
                    nc.vector.tensor_mul(t2, pmq, sin2q[:, csl])
                    nc.vector.tensor_add(qt_pair[t][:, csl], t1, t2)
                if t == 0:
                    emit_scores(1, 0)
                    emit_scores(2, 0)
                    # gate scaling + denominator column; emitted after the
                    # t=0 ropes so the wcol DRAM bounce never stalls them
                    for st in range(NT):
                        nc.vector.tensor_scalar(vaug[:, st, 0:HD],
                                                vraw[:, st, :],
                                                wcol[:, st:st + 1], None,
                                                mybir.AluOpType.mult)
                        nc.vector.tensor_copy(vaug[:, st, HD:HD + 1],
                                              wcol[:, st:st + 1])

            # interleaved block schedule: exp-heavy blocks mid-stream,
            # cheapest chunk (c=0) last to minimize the drain tail.
            seq = [("av", 1, 0), ("sc", 3, 0), ("av", 2, 0), ("sc", 3, 1),
                   ("av", 3, 0), ("sc", 1, 1), ("av", 1, 1), ("sc", 2, 1),
                   ("av", 2, 1), ("sc", 0, 0), ("av", 3, 1), ("sc", 0, 1),
                   ("av", 0, 0), ("av", 0, 1)]
            # each chunk's oproj is split in two: the second half is
            # deferred to the start of the NEXT schedule unit, threading
            # dense real PE work into the exp-paced (and otherwise
            # gap-prone) head of that unit instead of one 8us burst
            pair_done = {c: 0 for c in range(NCH)}
            pending = []
            for kind, c, t in seq:
                if pending:
                    pending.pop(0)()
                if kind == "sc":
                    emit_scores(c, t)
                else:
                    emit_av(c, 2 * t)
                    emit_av(c, 2 * t + 1)
                    pair_done[c] += 1
                    if pair_done[c] == HLOC // 2:
                        emit_oproj(c, tail=(c != 1),
                                   i_list=range(4 * c, 4 * c + 2))
                        pending.append(
                            lambda cc=c: emit_oproj(
                                cc, tail=(cc != 1),
                                i_list=range(4 * cc + 2, 4 * cc + 4)))
            for f in pending:
                f()

    _split_sync_waits(nc)
    return nc


def kernel(**inputs):
    global LAST_RESULT
    inp = {k: np.asarray(v) for k, v in inputs.items()}
    h = inp["hidden_states"].astype(F32).reshape(S, D)
    mask = inp["attention_mask"].astype(F32).reshape(S, S)
    cos = inp["cos"].astype(F32)
    sin = inp["sin"].astype(F32)
    Wf = inp["Wf"].astype(F32)
    W1 = inp["W1"].astype(F32)
    b1 = inp["b1"].astype(F32)
    W2 = inp["W2"].astype(F32)
    b2 = float(inp["b2"].reshape(-1)[0])
    gate_scale = float(inp["gate_scale"])
    Wq = inp["Wq"].astype(F32)
    Wk = inp["Wk"].astype(F32)
    Wv = inp["Wv"].astype(F32)
    Wo = inp["Wo"].astype(F32)

    maskT = np.ascontiguousarray(mask.T)
    mb, patterns, av_incl = _analyze_mask(maskT)
    n_pat = len(patterns)
    assert n_pat <= 64, f"too many unique mask patterns ({n_pat})"

    field_scale = float(F32(1.0 - ALPHA))
    b2_scaled = float(F32(b2) * F32(field_scale))

    nc = _build_program(mb, n_pat, av_incl, field_scale, b2_scaled, gate_scale)

    # host-side shared tensors
    hT = np.ascontiguousarray(h.T).astype(BF16)
    cosT = np.ascontiguousarray(cos.T)                       # [64, S]
    sinT = np.ascontiguousarray(sin.T)
    sin_signed = sinT.copy()
    sin_signed[0:32] = -sin_signed[0:32]
    inv_sqrt_hd = 1.0 / math.sqrt(HD)
    cos2q = np.vstack([cosT, cosT]) * inv_sqrt_hd            # [128, S]
    sin2q = np.vstack([sin_signed, sin_signed]) * inv_sqrt_hd
    cosk = cosT.astype(BF16)
    sink = sin_signed.astype(BF16)
    # rotate-half permutation (swap 32-row halves within each 64-row head)
    pq = np.zeros((P, P), dtype=BF16)
    for m in range(P):
        base = (m // HD) * HD
        r = m - base
        src = base + (r + 32) % HD
        pq[src, m] = 1.0
    w1a = (W1[:D].astype(np.float64)
           + Wf.astype(np.float64) @ W1[D:].astype(np.float64)).astype(F32).astype(BF16)

    def kmajor(w):
        # [D, F] -> [128, NT*F]: partition-major so the SBUF load is contiguous
        f = w.shape[1]
        return np.ascontiguousarray(
            w.reshape(NT, P, f).transpose(1, 0, 2).reshape(P, NT * f))

    w1a = kmajor(w1a)
    w2 = W2.reshape(64, 1).astype(BF16)
    b1c = b1.reshape(64, 1).astype(F32)
    pm = np.stack(patterns) if n_pat else None

    in_maps = []
    for c in range(NCORES):
        m = {
            "hT": hT,
            "wq": kmajor(Wq[:, c * HLOC * HD:(c + 1) * HLOC * HD].astype(BF16)),
            "wkv": kmajor(np.concatenate(
                [Wk[:, c * HD:(c + 1) * HD], Wv[:, c * HD:(c + 1) * HD]],
                axis=1).astype(BF16)),
            "wo": np.ascontiguousarray(
                Wo[c * HLOC * HD:(c + 1) * HLOC * HD, :].astype(BF16)
                .reshape(2, P, D).transpose(1, 0, 2).reshape(P, 2 * D)),
            "w1a": w1a, "w2": w2, "b1": b1c,
            "cos2q": cos2q.astype(BF16), "sin2q": sin2q.astype(BF16),
            "cosk": cosk, "sink": sink, "pq": pq,
        }
        if n_pat:
            m["pmask"] = pm
        in_maps.append(m)

    trace = False
    if os.environ.get("KERNEL_TRACE"):
        try:
            import antenv.axon_hooks  # noqa: F401  (profiling shim, dev only)
            trace = True
        except ImportError:
            pass

    res = run_bass_kernel_spmd(nc, in_maps, list(range(NCORES)), trace=trace)
    LAST_RESULT = res

    out = np.zeros((S, D), dtype=F32)
    for c in range(NCORES):
        out += res.results[c]["out"].astype(F32)
    return out.reshape(1, S, D)

